# revision 1
# baseline (speedup 1.0000x reference)
"""Trainium2 Bass kernel for EnhancedMultiHeadAttention (B=2, S=2048, DM=1024, H=16).

Sharding: 8 NeuronCores = 2 batches x 4 query-row blocks of 512 rows. Each
core computes K/V for its whole batch (4x redundant; cheaper than the
~30us-per-op AllGather firmware cost measured on this runtime), plus
attention, output projection, gate and layernorm for its own 512 query rows.
No collectives; the host concatenates the 8 output shards.

Schedule: one fused window. Q^T is projected up front; each head pair's K
rows are projected just-in-time; V column halves are projected into
SBUF-resident tiles, with pair 0's V tiles fused into its attention t-loop
so the ScalarE GELU stream (the ~148us serial bottleneck: 16.8M exact-erf
GELU elements/core at 1 elem/cycle/lane) starts as early as possible. All
projection matmuls fill PE slack underneath the GELU stream, which also
keeps the PE HAM clock un-throttled. The out-projection (both orientations:
row-major for the residual/LN epilogue, transposed as lhsT for the gate
matmul), gate, sigmoid, and the bn_stats-based layernorm trail the window,
pipelined per 128-row tile.

Precision: all matmuls run in fp32r (1 cycle/row at N>=256, ~1.5e-4 per
matmul) except attn @ v in bf16, which lets the two heads of a pair be
col-packed into one PSUM bank (fp32r cannot target PSUM partitions 64-127;
row-packed K=64 score matmuls are fine). The 1/sqrt(64) score scale rides
the GELU activation's free affine; softmax(attention_weights) is folded
into Wv/bv on the host; per-core query columns are permuted to the front of
x^T so Q projects from the same resident tile (t-order in attention is
permutation-invariant as long as K and V share it).

Measured on 8 axon-tunneled trn2 cores: HW exec ~382us, rel err 2.4e-4
(vs fp32 reference; fp64-reference check identical).
"""
import math
import os
import sys

import numpy as np

for _p in ("/opt/trn_rl_repo", "/opt/pypackages"):
    if _p not in sys.path:
        sys.path.append(_p)

import concourse.bass as bass
import concourse.mybir as mybir
import concourse.tile as tile
from concourse import bacc
from concourse.bass_utils import run_bass_kernel_spmd

F32R = mybir.dt.float32r
F32 = mybir.dt.float32
BF16 = mybir.dt.bfloat16
AF = mybir.ActivationFunctionType
ALU = mybir.AluOpType

B, S, DM, H = 2, 2048, 1024, 16
HD = DM // H                  # 64
SQ = 512                      # query rows per core
NP = 128                      # partitions
KC = DM // NP                 # 8 contraction chunks
NT = S // NP                  # 16 key/value tiles
NPAIR = H // 2                # 8 head pairs
NST = SQ // NP                # 4 row tiles in row-layout phases
N512 = 512
NQ = 512                      # v-projection column half width
SCALE = 1.0 / math.sqrt(HD)
EPS = 1e-5

_CACHE = {}
_TRACE = [False]
_LAST_RESULT = [None]


def _bcast(ap_1d, p=NP):
    return bass.AP(tensor=ap_1d.tensor, offset=ap_1d.offset,
                   ap=[[0, p]] + list(ap_1d.ap))


def _build():
    nc = bacc.Bacc("TRN2", target_bir_lowering=False, debug=False)

    xT_d = nc.dram_tensor("xT", [DM, S], F32R, kind="ExternalInput").ap()
    xr_d = nc.dram_tensor("xr", [SQ, DM], F32, kind="ExternalInput").ap()
    wkT_d = nc.dram_tensor("wkT", [DM, DM], F32R, kind="ExternalInput").ap()
    wvT_d = nc.dram_tensor("wvT", [DM, DM], F32R, kind="ExternalInput").ap()
    wqT_d = nc.dram_tensor("wqT", [DM, DM], F32R, kind="ExternalInput").ap()
    woT_d = nc.dram_tensor("woT", [DM, DM], F32R, kind="ExternalInput").ap()
    wgT_d = nc.dram_tensor("wgT", [DM, DM], F32R, kind="ExternalInput").ap()
    bq_d = nc.dram_tensor("bq", [DM], F32, kind="ExternalInput").ap()
    bk_d = nc.dram_tensor("bk", [DM], F32, kind="ExternalInput").ap()
    bv_d = nc.dram_tensor("bv", [DM], F32, kind="ExternalInput").ap()
    bo_d = nc.dram_tensor("bo", [DM], F32, kind="ExternalInput").ap()
    bg_d = nc.dram_tensor("bg", [DM], F32, kind="ExternalInput").ap()
    gam_d = nc.dram_tensor("gam", [DM], F32, kind="ExternalInput").ap()
    bet_d = nc.dram_tensor("bet", [DM], F32, kind="ExternalInput").ap()
    y_d = nc.dram_tensor("y", [SQ, DM], F32, kind="ExternalOutput").ap()

    xT_v = xT_d.rearrange("(c p) s -> p c s", p=NP)
    wk_v = wkT_d.rearrange("(c p) d -> p c d", p=NP)
    wv_v = wvT_d.rearrange("(c p) d -> p c d", p=NP)
    wq_v = wqT_d.rearrange("(c p) d -> p c d", p=NP)
    wo_v = woT_d.rearrange("(c p) d -> p c d", p=NP)
    wg_v = wgT_d.rearrange("(c p) d -> p c d", p=NP)

    with tile.TileContext(nc) as tc:
        with tc.tile_pool(name="pers", bufs=1) as pers, \
             tc.tile_pool(name="acc", bufs=1) as acc:
            bq_sb = pers.tile([NP, KC], F32)
            bk_sb = pers.tile([NP, KC], F32)
            bo_sb = pers.tile([NP, KC], F32)
            nc.sync.dma_start(out=bq_sb, in_=bq_d.rearrange("(c p) -> p c", p=NP))
            nc.sync.dma_start(out=bk_sb, in_=bk_d.rearrange("(c p) -> p c", p=NP))
            nc.sync.dma_start(out=bo_sb, in_=bo_d.rearrange("(c p) -> p c", p=NP))
            bv_bc = pers.tile([NP, DM], F32)
            nc.sync.dma_start(out=bv_bc, in_=_bcast(bv_d))
            eps_sb = pers.tile([NP, 1], F32)
            nc.vector.memset(eps_sb, EPS)

            ctxT_sb = acc.tile([NP, NPAIR, SQ], F32R)

            with tc.tile_pool(name="xres", bufs=1) as xres, \
                 tc.tile_pool(name="qres", bufs=1) as qres, \
                 tc.tile_pool(name="wvp", bufs=1) as wvp, \
                 tc.tile_pool(name="wsl", bufs=2) as wsl, \
                 tc.tile_pool(name="kpp", bufs=2) as kpp, \
                 tc.tile_pool(name="vqp", bufs=2) as vqp, \
                 tc.tile_pool(name="attp", bufs=3) as attp, \
                 tc.tile_pool(name="pp", bufs=2, space="PSUM") as pp, \
                 tc.tile_pool(name="scop", bufs=2, space="PSUM") as scop, \
                 tc.tile_pool(name="cxp", bufs=2, space="PSUM") as cxp:
                xT_sb = xres.tile([NP, KC, S], F32R)
                qT_sb = qres.tile([NP, KC, SQ], F32R)

                v_q = [None] * 2

                wv_sbs = [None] * 2

                def v_open(q):
                    wv_sb = wvp.tile([NP, KC, NQ], F32R, tag="wv", name="wv_sb")
                    nc.gpsimd.dma_start(out=wv_sb,
                                        in_=wv_v[:, :, q * NQ:(q + 1) * NQ])
                    wv_sbs[q] = wv_sb
                    vq = vqp.tile([NP, NT, NQ], BF16, tag="vq", name="vq")
                    v_q[q] = vq

                def v_tt(q, tt):
                    ps_t = pp.tile([NP, NQ], F32, tag="pj", name="ps_t")
                    for kc in range(KC):
                        nc.tensor.matmul(
                            ps_t,
                            xT_sb[:, kc, tt * NP:(tt + 1) * NP],
                            wv_sbs[q][:, kc, :],
                            start=(kc == 0), stop=(kc == KC - 1))
                    nc.vector.tensor_add(
                        v_q[q][:, tt, :], ps_t, bv_bc[:, q * NQ:(q + 1) * NQ])

                kpairs = [None] * NPAIR

                wk_tiles = {}

                def wk_dma(p):
                    wk_sl = wsl.tile([NP, KC, NP], F32R, tag="wk", name="wk_sl")
                    nc.sync.dma_start(out=wk_sl,
                                      in_=wk_v[:, :, p * NP:(p + 1) * NP])
                    wk_tiles[p] = wk_sl

                def prepare(p):
                    # kpair[d, t] = sum_k Wk[d, k] x[t, k] + bk[d], d in pair rows
                    if p not in wk_tiles:
                        wk_dma(p)
                    wk_sl = wk_tiles[p]
                    kpair = kpp.tile([NP, S], F32R, tag="kp", name="kpair")
                    for ts in range(S // N512):
                        ps_t = pp.tile([NP, N512], F32, tag="pj", name="ps_t")
                        for kc in range(KC):
                            nc.tensor.matmul(
                                ps_t,
                                wk_sl[:, kc, :],
                                xT_sb[:, kc, ts * N512:(ts + 1) * N512],
                                start=(kc == 0), stop=(kc == KC - 1))
                        nc.vector.tensor_scalar_add(
                            kpair[:, ts * N512:(ts + 1) * N512], ps_t,
                            bk_sb[:, p:p + 1])
                    kpairs[p] = kpair

                def attn(p, pre_t=None):
                    kpair = kpairs[p]
                    vq = v_q[p // 4]
                    c0 = (p % 4) * NP
                    ctx_ps = cxp.tile([NP, SQ], F32, tag="cx", name="ctx_ps")
                    for t in range(NT):
                        if pre_t is not None:
                            pre_t(t)
                        sco = scop.tile([NP, 2 * SQ], F32, tag="sc", name="sco")
                        nc.tensor.matmul(sco[:, 0:SQ],
                                         kpair[0:64, t * NP:(t + 1) * NP],
                                         qT_sb[0:64, p, :],
                                         start=True, stop=True,
                                         tile_position=(0, 0))
                        nc.tensor.matmul(sco[:, SQ:2 * SQ],
                                         kpair[64:128, t * NP:(t + 1) * NP],
                                         qT_sb[64:128, p, :],
                                         start=True, stop=True,
                                         tile_position=(64, 0))
                        att_t = attp.tile([NP, 2 * SQ], BF16, tag="at", name="att_t")
                        nc.scalar.activation(out=att_t, in_=sco, func=AF.Gelu,
                                             scale=SCALE)
                        nc.tensor.matmul(ctx_ps[0:64, :], vq[:, t, c0:c0 + 64],
                                         att_t[:, 0:SQ],
                                         start=(t == 0), stop=(t == NT - 1),
                                         tile_position=(0, 0))
                        nc.tensor.matmul(ctx_ps[64:128, :], vq[:, t, c0 + 64:c0 + NP],
                                         att_t[:, SQ:2 * SQ],
                                         start=(t == 0), stop=(t == NT - 1),
                                         tile_position=(0, 64))
                    nc.vector.tensor_copy(ctxT_sb[:, p, :], ctx_ps)

                # opening DMA order: wk(0), xT query-cols chunk, then the
                # Q projection (wq slices land right behind), then rest of xT
                wk_dma(0)
                for kc in range(KC):
                    nc.sync.dma_start(
                        out=xT_sb[:, kc, 0:N512], in_=xT_v[:, kc, 0:N512])
                for dt in range(KC):
                    wq_sl = wsl.tile([NP, KC, NP], F32R, tag="wq", name="wq_sl")
                    nc.sync.dma_start(out=wq_sl,
                                      in_=wq_v[:, :, dt * NP:(dt + 1) * NP])
                    ps_q = pp.tile([NP, SQ], F32, tag="pj", name="ps_q")
                    for kc in range(KC):
                        nc.tensor.matmul(ps_q, wq_sl[:, kc, :],
                                         xT_sb[:, kc, 0:SQ],
                                         start=(kc == 0), stop=(kc == KC - 1))
                    nc.vector.tensor_scalar_add(qT_sb[:, dt, :], ps_q,
                                                bq_sb[:, dt:dt + 1])
                for ts in range(1, S // N512):
                    for kc in range(KC):
                        nc.sync.dma_start(
                            out=xT_sb[:, kc, ts * N512:(ts + 1) * N512],
                            in_=xT_v[:, kc, ts * N512:(ts + 1) * N512])

                prepare(0)
                v_open(0)
                attn(0, pre_t=lambda t: v_tt(0, t))
                prepare(1)
                v_open(1)
                for _tt in range(0, 4):
                    v_tt(1, _tt)
                attn(1)
                prepare(2)
                for _tt in range(4, 8):
                    v_tt(1, _tt)
                attn(2)
                prepare(3)
                for _tt in range(8, 12):
                    v_tt(1, _tt)
                attn(3)
                prepare(4)
                for _tt in range(12, NT):
                    v_tt(1, _tt)
                attn(4)
                prepare(5)
                attn(5)
                prepare(6)
                attn(6)
                prepare(7)
                attn(7)

            # ------------- out proj, gate + epilogue -----------------------
            with tc.tile_pool(name="w2", bufs=2) as w2, \
                 tc.tile_pool(name="big", bufs=1) as big, \
                 tc.tile_pool(name="pp2", bufs=4, space="PSUM") as pp2:
                wo_sb = w2.tile([NP, KC, DM], F32R, tag="w2t", name="wo_sb")
                for dt in range(KC):
                    nc.sync.dma_start(out=wo_sb[:, :, dt * NP:(dt + 1) * NP],
                                      in_=wo_v[:, :, dt * NP:(dt + 1) * NP])
                bo_bc = big.tile([NP, DM], F32)
                nc.sync.dma_start(out=bo_bc, in_=_bcast(bo_d))
                outT_sb = big.tile([NP, KC, SQ], F32R)
                for dt in range(KC):
                    ps_t = pp2.tile([NP, SQ], F32, tag="po", name="ps_t")
                    for dc in range(KC):
                        nc.tensor.matmul(
                            ps_t,
                            wo_sb[:, dc, dt * NP:(dt + 1) * NP],
                            ctxT_sb[:, dc, :],
                            start=(dc == 0), stop=(dc == KC - 1))
                    nc.vector.tensor_scalar_add(outT_sb[:, dt, :], ps_t,
                                                bo_sb[:, dt:dt + 1])
                orow_sb = big.tile([NP, NST, DM], F32)
                wg_sb = w2.tile([NP, KC, DM], F32R, tag="w2t", name="wg_sb")
                for dt in range(KC):
                    nc.sync.dma_start(out=wg_sb[:, :, dt * NP:(dt + 1) * NP],
                                      in_=wg_v[:, :, dt * NP:(dt + 1) * NP])
                bg_bc = big.tile([NP, DM], F32)
                gam_bc = big.tile([NP, DM], F32)
                bet_bc = big.tile([NP, DM], F32)
                nc.sync.dma_start(out=bg_bc, in_=_bcast(bg_d))
                nc.sync.dma_start(out=gam_bc, in_=_bcast(gam_d))
                nc.sync.dma_start(out=bet_bc, in_=_bcast(bet_d))
                xr_sb = big.tile([NP, NST, DM], F32)
                nc.sync.dma_start(out=xr_sb,
                                  in_=xr_d.rearrange("(n p) d -> p n d", p=NP))

                gate_sb = big.tile([NP, NST, DM], F32)
                t1_sb = big.tile([NP, NST, DM], F32)
                stats = pers.tile([NP, 2, 6], F32)
                mv = pers.tile([NP, 2], F32)
                std = pers.tile([NP, 1], F32)
                rstd = pers.tile([NP, 1], F32)
                y_sb = gate_sb
                for st in range(NST):
                    for ns in range(DM // N512):
                        ps_t = pp2.tile([NP, N512], F32, tag="po", name="ps_t")
                        for dc in range(KC):
                            nc.tensor.matmul(
                                ps_t,
                                outT_sb[:, dc, st * NP:(st + 1) * NP],
                                wg_sb[:, dc, ns * N512:(ns + 1) * N512],
                                start=(dc == 0), stop=(dc == KC - 1))
                        nc.vector.tensor_add(
                            gate_sb[:, st, ns * N512:(ns + 1) * N512],
                            ps_t, bg_bc[:, ns * N512:(ns + 1) * N512])
                    nc.scalar.activation(out=gate_sb[:, st, :],
                                         in_=gate_sb[:, st, :], func=AF.Sigmoid)
                    for ns in range(DM // N512):
                        ps_t = pp2.tile([NP, N512], F32, tag="po", name="ps_t")
                        for dc in range(KC):
                            nc.tensor.matmul(
                                ps_t,
                                ctxT_sb[:, dc, st * NP:(st + 1) * NP],
                                wo_sb[:, dc, ns * N512:(ns + 1) * N512],
                                start=(dc == 0), stop=(dc == KC - 1))
                        nc.vector.tensor_add(
                            orow_sb[:, st, ns * N512:(ns + 1) * N512],
                            ps_t, bo_bc[:, ns * N512:(ns + 1) * N512])
                    # y_pre = gate*(out - x) + 2x
                    nc.vector.tensor_sub(t1_sb[:, st, :], orow_sb[:, st, :],
                                         xr_sb[:, st, :])
                    nc.vector.tensor_mul(orow_sb[:, st, :], t1_sb[:, st, :],
                                         gate_sb[:, st, :])
                    nc.vector.scalar_tensor_tensor(
                        out=t1_sb[:, st, :], in0=xr_sb[:, st, :], scalar=2.0,
                        in1=orow_sb[:, st, :], op0=ALU.mult, op1=ALU.add)
                    # layernorm over DM
                    yv = t1_sb[:, st, :].rearrange("p (g d) -> p g d", g=2)
                    for g in range(2):
                        nc.vector.bn_stats(out=stats[:, g, :], in_=yv[:, g, :])
                    nc.vector.bn_aggr(out=mv, in_=stats)
                    nc.scalar.activation(out=std, in_=mv[:, 1:2], func=AF.Sqrt,
                                         bias=eps_sb)
                    nc.vector.reciprocal(rstd, std)
                    nc.vector.tensor_scalar(
                        out=orow_sb[:, st, :], in0=t1_sb[:, st, :],
                        scalar1=mv[:, 0:1], scalar2=rstd,
                        op0=ALU.subtract, op1=ALU.mult)
                    nc.vector.tensor_mul(orow_sb[:, st, :], orow_sb[:, st, :],
                                         gam_bc)
                    nc.vector.tensor_add(y_sb[:, st, :], orow_sb[:, st, :],
                                         bet_bc)
                    nc.sync.dma_start(
                        out=y_d.rearrange("(n p) d -> p n d", p=NP)[:, st, :],
                        in_=y_sb[:, st, :])

    nc.compile()
    return nc


def kernel(x, Wq, bq, Wk, bk, Wv, bv, Wo, bo, Wg, bg, attention_weights,
           ln_gamma, ln_beta):
    x = np.asarray(x, dtype=np.float32)
    f32 = lambda a: np.ascontiguousarray(np.asarray(a, dtype=np.float32))
    Wq, Wk, Wv, Wo, Wg = map(f32, (Wq, Wk, Wv, Wo, Wg))
    bq, bk, bv, bo, bg = map(f32, (bq, bk, bv, bo, bg))
    aw, gam, bet = map(f32, (attention_weights, ln_gamma, ln_beta))

    if "nc" not in _CACHE:
        _CACHE["nc"] = _build()
    nc = _CACHE["nc"]

    # fold softmax(attention_weights) into Wv / bv
    e = np.exp(aw - aw.max())
    head_w = (e / e.sum()).astype(np.float32)
    hw_exp = np.repeat(head_w, HD)              # [DM]
    Wv_s = Wv * hw_exp[:, None]
    bv_s = bv * hw_exp

    wqT = np.ascontiguousarray(Wq.T)
    wkT = np.ascontiguousarray(Wk.T)
    wvT = np.ascontiguousarray(Wv_s.T)
    woT = np.ascontiguousarray(Wo.T)
    wgT = np.ascontiguousarray(Wg.T)

    in_maps = []
    for c in range(8):
        b, blk = divmod(c, 4)
        r0 = blk * SQ
        xb = x[b]
        perm = np.r_[r0:r0 + SQ, 0:r0, r0 + SQ:S]
        in_maps.append({
            "xT": np.ascontiguousarray(xb[perm].T),
            "xr": np.ascontiguousarray(xb[r0:r0 + SQ]),
            "wkT": wkT, "wvT": wvT, "wqT": wqT, "woT": woT, "wgT": wgT,
            "bq": bq, "bk": bk, "bv": bv_s, "bo": bo, "bg": bg,
            "gam": gam, "bet": bet,
        })

    last_exc = None
    for _attempt in range(3):
        try:
            res = run_bass_kernel_spmd(nc, in_maps, core_ids=list(range(8)),
                                       trace=_TRACE[0])
            break
        except Exception as exc:  # flaky NRT_EXEC_UNIT errors: retry
            last_exc = exc
            import time
            time.sleep(2.0)
    else:
        raise last_exc
    _LAST_RESULT[0] = res

    y = np.empty((B, S, DM), dtype=np.float32)
    for c in range(8):
        b, blk = divmod(c, 4)
        r0 = blk * SQ
        y[b, r0:r0 + SQ] = res.results[c]["y"]
    return y

